# revision 11
# baseline (speedup 1.0000x reference)
"""Trainium2 Bass kernel for nn_MultiHeadAttention_9131100471609.

8-core SPMD: core c handles batch b=c//2, query-half qh=c%2 (1024 queries,
all 8 heads).  No collectives; host pre-transposes inputs and reassembles.

Per-core pipeline (feature-major activations, RoPE pairs de-interleaved via
a host-side permutation of the Wq/Wk rows):
  1. Q/K/V projections:  out^T = W'.X^T  (PSUM accumulate over f-chunks),
     bias+relu epilogue on DVE (tensor_scalar add+max).
  2. RoPE: partition-swap via 4 SBUF->SBUF DMAs, then 3 tensor_tensor ops
     per 128-row chunk against host-precomputed cos/sin tables.
  3. Attention per head-pair (2 heads row-packed in the PE at partition
     bases 0/64):  scoresT (k,q) matmuls -> one ACT exp per 2-bank PSUM
     tile -> AV accumulation with ones-augmented V (softmax sums free).
  4. Deferred softmax normalization: s rows -> reciprocal -> DMA broadcast
     -> TT multiply into Y^T.
  5. Output projection with rank-1 bias matmul + ACT relu.
"""

import sys

for _p in ("/opt/trn_rl_repo", "/root/.axon_site/_ro/trn_rl_repo"):
    if _p not in sys.path:
        sys.path.append(_p)

import math

import numpy as np

S, B, F, H = 2048, 4, 512, 8
D = F // H  # 64
THETA = 10000.0
NCORES = 8

_CACHE = {}


def _build(S_=S, Sq_=None):
    """Build the Bass module.  Parametrized by sequence length so tests can
    simulate a reduced-size variant.  Sq_ = per-core query count."""
    import concourse.mybir as mybir
    import concourse.tile as tile
    from concourse import bacc
    from concourse.bass import AP

    if Sq_ is None:
        Sq_ = S_ // 2
    f32 = mybir.dt.float32
    AF = mybir.ActivationFunctionType
    ALU = mybir.AluOpType

    KC = S_ // 128      # k-chunks
    TCH = S_ // 128     # v token-chunks
    QB = Sq_ // 512     # 512-wide q blocks
    QN = Sq_            # query tokens per core
    SCALE = 1.0 / math.sqrt(D)

    nc = bacc.Bacc("TRN2", target_bir_lowering=False, debug=False)

    # ---- DRAM I/O ----
    xqT = nc.dram_tensor("xqT", (F, QN), f32, kind="ExternalInput")
    xkT = nc.dram_tensor("xkT", (F, S_), f32, kind="ExternalInput")
    xvT = nc.dram_tensor("xvT", (F, S_), f32, kind="ExternalInput")
    wqT = nc.dram_tensor("wqT", (F, F), f32, kind="ExternalInput")
    wkT = nc.dram_tensor("wkT", (F, F), f32, kind="ExternalInput")
    wvT = nc.dram_tensor("wvT", (F, F), f32, kind="ExternalInput")
    woT = nc.dram_tensor("woT", (F, F), f32, kind="ExternalInput")
    bq = nc.dram_tensor("bq", (F, 1), f32, kind="ExternalInput")
    bk = nc.dram_tensor("bk", (F, 1), f32, kind="ExternalInput")
    bv_row = nc.dram_tensor("bv_row", (1, F), f32, kind="ExternalInput")
    bo_row = nc.dram_tensor("bo_row", (1, F), f32, kind="ExternalInput")
    ropeC = nc.dram_tensor("ropeC", (128, S_), f32, kind="ExternalInput")
    ropeS = nc.dram_tensor("ropeS", (128, S_), f32, kind="ExternalInput")
    out_d = nc.dram_tensor("out", (QN, F), f32, kind="ExternalOutput")

    qoff = None  # query token offset within [0,S) is qh*Sq_; rope table cols
    # are selected host-side by passing a pre-sliced ropeCq/ropeSq?  No - the
    # same table works: q tokens are at absolute positions qh*Sq_+t, so the
    # kernel needs the q-slice of the tables.  Pass them separately:
    ropeCq = nc.dram_tensor("ropeCq", (128, QN), f32, kind="ExternalInput")
    ropeSq = nc.dram_tensor("ropeSq", (128, QN), f32, kind="ExternalInput")

    with tile.TileContext(nc) as tc:
        # ---------- persistent pools ----------
        with (
            tc.tile_pool(name="qkv", bufs=1) as p_qkv,       # qT/kT chunks
            tc.tile_pool(name="vsb", bufs=1) as p_vsb,       # V_aug tiles
            tc.tile_pool(name="yt", bufs=1) as p_yt,         # normalized Y^T
            tc.tile_pool(name="small", bufs=1) as p_small,   # rows, tables
            tc.tile_pool(name="wo", bufs=1) as p_wo,
        ):
            ones1 = p_small.tile([1, 128], f32)
            nc.vector.memset(ones1[:], 1.0)
            tbo = p_small.tile([1, F], f32)
            nc.sync.dma_start(tbo[:], bo_row[:])
            tropeC = p_small.tile([128, S_], f32)
            nc.sync.dma_start(tropeC[:], ropeC[:])
            tropeS = p_small.tile([128, S_], f32)
            nc.sync.dma_start(tropeS[:], ropeS[:])
            tropeCq = p_small.tile([128, QN], f32)
            nc.sync.dma_start(tropeCq[:], ropeCq[:])
            tropeSq = p_small.tile([128, QN], f32)
            nc.sync.dma_start(tropeSq[:], ropeSq[:])
            two = [p_wo.tile([128, F], f32, name=f"two{i}", tag=f"two{i}") for i in range(4)]
            for fc in range(4):
                nc.sync.dma_start(two[fc][:], woT[fc * 128:(fc + 1) * 128, :])

            qT = [p_qkv.tile([128, QN], f32, name=f"qT{i}", tag=f"qT{i}") for i in range(4)]
            kT = [p_qkv.tile([128, S_], f32, name=f"kT{i}", tag=f"kT{i}") for i in range(4)]
            vA = [p_vsb.tile([128, 8 * (D + 1)], f32, name=f"vA{i}", tag=f"vA{i}") for i in range(TCH)]
            yT = [p_yt.tile([128, QN], f32, name=f"yT{i}", tag=f"yT{i}") for i in range(4)]

            # ---------- phase 1: projections + rope ----------
            with (
                tc.tile_pool(name="xin", bufs=6) as p_x,
                tc.tile_pool(name="wts", bufs=1) as p_w,
                tc.tile_pool(name="brow", bufs=1) as p_b,
                tc.tile_pool(name="ptmp", bufs=2) as p_t,
                tc.tile_pool(name="psproj", bufs=4, space="PSUM") as ps_pr,
            ):
                twq = [p_w.tile([128, F], f32, name=f"twq{i}", tag=f"twq{i}") for i in range(4)]
                twk = [p_w.tile([128, F], f32, name=f"twk{i}", tag=f"twk{i}") for i in range(4)]
                twv = [p_w.tile([128, F], f32, name=f"twv{i}", tag=f"twv{i}") for i in range(4)]
                for fc in range(4):
                    sl = slice(fc * 128, (fc + 1) * 128)
                    nc.sync.dma_start(twq[fc][:], wqT[sl, :])
                    nc.sync.dma_start(twk[fc][:], wkT[sl, :])
                    nc.sync.dma_start(twv[fc][:], wvT[sl, :])
                tbq = [p_b.tile([128, 1], f32, name=f"tbq{i}", tag=f"tbq{i}") for i in range(4)]
                tbk = [p_b.tile([128, 1], f32, name=f"tbk{i}", tag=f"tbk{i}") for i in range(4)]
                for gc in range(4):
                    nc.sync.dma_start(tbq[gc][:], bq[gc * 128:(gc + 1) * 128, :])
                    nc.sync.dma_start(tbk[gc][:], bk[gc * 128:(gc + 1) * 128, :])
                tbv = p_b.tile([1, F], f32)
                nc.sync.dma_start(tbv[:], bv_row[:])

                def load_xblock(xdram, nb):
                    """Column block nb (512 tokens) of an (F, ntok) input."""
                    tiles = []
                    for fc in range(4):
                        tx = p_x.tile([128, 512], f32, name="xs", tag="xs")
                        nc.sync.dma_start(
                            tx[:], xdram[fc * 128:(fc + 1) * 128,
                                         nb * 512:(nb + 1) * 512])
                        tiles.append(tx)
                    return tiles

                def rope_inplace(x, cosT, sinT, ncols):
                    """x = x*cos + swap32(x)*sin  (x: (128, ncols) AP)."""
                    sw = p_t.tile([128, S_], f32, tag="ropesw")
                    for blk in range(4):
                        sb_ = blk ^ 1
                        nc.sync.dma_start(
                            sw[blk * 32:(blk + 1) * 32, 0:ncols],
                            x[sb_ * 32:(sb_ + 1) * 32, :])
                    t2 = p_t.tile([128, S_], f32, tag="ropet2", bufs=1)
                    nc.vector.tensor_mul(t2[0:128, 0:ncols], sw[0:128, 0:ncols], sinT)
                    nc.vector.tensor_mul(x, x, cosT)
                    nc.vector.tensor_add(x, x, t2[0:128, 0:ncols])

                # Q projection (feature-major out into qT), then rope
                for nb in range(QN // 512):
                    xb = load_xblock(xqT, nb)
                    for gc in range(4):
                        pq = ps_pr.tile([128, 512], f32, tag="pp")
                        for fc in range(4):
                            nc.tensor.matmul(
                                pq[:],
                                twq[fc][:, gc * 128:(gc + 1) * 128],
                                xb[fc][:],
                                start=(fc == 0), stop=(fc == 3))
                        nc.vector.tensor_scalar(
                            qT[gc][:, nb * 512:(nb + 1) * 512], pq[:],
                            tbq[gc][:], 0.0, ALU.add, ALU.max)
                # K projection
                for nb in range(S_ // 512):
                    xb = load_xblock(xkT, nb)
                    for gc in range(4):
                        pk = ps_pr.tile([128, 512], f32, tag="pp")
                        for fc in range(4):
                            nc.tensor.matmul(
                                pk[:],
                                twk[fc][:, gc * 128:(gc + 1) * 128],
                                xb[fc][:],
                                start=(fc == 0), stop=(fc == 3))
                        nc.vector.tensor_scalar(
                            kT[gc][:, nb * 512:(nb + 1) * 512], pk[:],
                            tbk[gc][:], 0.0, ALU.add, ALU.max)
                # rope
                for gc in range(4):
                    rope_inplace(qT[gc][:], tropeCq[:], tropeSq[:], QN)
                    rope_inplace(kT[gc][:], tropeC[:], tropeS[:], S_)
                # V projection (token-major out, ones-augmented layout)
                for nb in range(S_ // 512):
                    xb = load_xblock(xvT, nb)
                    for sub in range(4):
                        tch = nb * 4 + sub
                        pv = ps_pr.tile([128, 512], f32, tag="pp")
                        for fc in range(4):
                            nc.tensor.matmul(
                                pv[:],
                                xb[fc][:, sub * 128:(sub + 1) * 128],
                                twv[fc][:],
                                start=(fc == 0), stop=False)
                        nc.tensor.matmul(
                            pv[:], ones1[:], tbv[:], start=False, stop=True)
                        va = vA[tch]
                        nc.vector.memset(va[:], 1.0)
                        dst = va[:].rearrange(
                            "p (h c) -> p h c", h=8)[:, :, 0:D]
                        src = pv[:].rearrange(
                            "p (h c) -> p h c", h=8)
                        nc.vector.tensor_scalar(
                            dst, src, 0.0, None, ALU.max)

            # ---------- phase 2: attention ----------
            with (
                tc.tile_pool(name="att", bufs=3) as p_a,
                tc.tile_pool(name="yun", bufs=5) as p_yun,
                tc.tile_pool(name="srows", bufs=2) as p_sr,
                tc.tile_pool(name="pssc", bufs=2, space="PSUM") as ps_sc,
                tc.tile_pool(name="psav", bufs=3, space="PSUM") as ps_av,
                tc.tile_pool(name="pspb", bufs=1, space="PSUM") as ps_pb,
            ):
                for qb in range(QB):
                    qs = slice(qb * 512, (qb + 1) * 512)
                    srow = p_sr.tile([1, 8 * 512], f32, tag="srow", bufs=1)
                    yun2s = []
                    for hp in range(4):
                        pav0 = ps_av.tile([D + 1, 512], f32, tag="pav")
                        pav1 = ps_av.tile([D + 1, 512], f32, tag="pav")
                        for kc in range(KC):
                            ks = slice(kc * 128, (kc + 1) * 128)
                            psc = ps_sc.tile([128, 1024], f32, tag="psc")
                            nc.tensor.matmul(
                                psc[:, 0:512],
                                kT[hp][0:64, ks], qT[hp][0:64, qs],
                                start=True, stop=True)
                            nc.tensor.matmul(
                                psc[:, 512:1024],
                                kT[hp][64:128, ks], qT[hp][64:128, qs],
                                start=True, stop=True)
                            a_t = p_a.tile([128, 1024], f32, tag="aexp")
                            nc.scalar.activation(
                                a_t[:], psc[:], AF.Exp, scale=SCALE)
                            nc.tensor.matmul(
                                pav0[:],
                                vA[kc][:, (2 * hp) * 65:(2 * hp) * 65 + 65],
                                a_t[:, 0:512],
                                start=(kc == 0), stop=(kc == KC - 1))
                            nc.tensor.matmul(
                                pav1[:],
                                vA[kc][:, (2 * hp + 1) * 65:(2 * hp + 1) * 65 + 65],
                                a_t[:, 512:1024],
                                start=(kc == 0), stop=(kc == KC - 1))
                        # extract s rows + unnormalized Y^T
                        nc.scalar.copy(
                            srow[0:1, (2 * hp) * 512:(2 * hp) * 512 + 512],
                            pav0[64:65, :])
                        nc.scalar.copy(
                            srow[0:1, (2 * hp + 1) * 512:(2 * hp + 1) * 512 + 512],
                            pav1[64:65, :])
                        yun2 = p_yun.tile([128, 512], f32, tag="yun")
                        nc.vector.tensor_copy(yun2[0:64, :], pav0[0:64, :])
                        nc.vector.tensor_copy(yun2[64:128, :], pav1[0:64, :])
                        yun2s.append(yun2)
                    # normalization for all 8 heads of this q-block:
                    # scatter s -> (8,512), reciprocal, gather back to a
                    # base-0 row, rank-1 matmul broadcast into PSUM, then
                    # one TT mult per head-pair (in1 reads PSUM directly).
                    sstack = p_sr.tile([8, 512], f32, tag="sstack")
                    for h in range(8):
                        nc.sync.dma_start(
                            sstack[h:h + 1, :],
                            srow[0:1, h * 512:(h + 1) * 512])
                    srec_st = p_sr.tile([8, 512], f32, tag="srec_st")
                    nc.vector.reciprocal(srec_st[:], sstack[:])
                    srec_row = p_sr.tile([1, 8 * 512], f32, tag="srec_row", bufs=1)
                    for h in range(8):
                        nc.sync.dma_start(
                            srec_row[0:1, h * 512:(h + 1) * 512],
                            srec_st[h:h + 1, :])
                    for hp in range(4):
                        pb = ps_pb.tile([128, 512], f32, tag="pb")
                        nc.tensor.matmul(
                            pb[0:64, :], ones1[0:1, 0:64],
                            srec_row[0:1, (2 * hp) * 512:(2 * hp) * 512 + 512],
                            start=True, stop=True)
                        nc.tensor.matmul(
                            pb[64:128, :], ones1[0:1, 0:64],
                            srec_row[0:1, (2 * hp + 1) * 512:(2 * hp + 1) * 512 + 512],
                            start=True, stop=True)
                        nc.vector.tensor_mul(
                            yT[hp][:, qs], yun2s[hp][:], pb[:])

            # ---------- phase 3: output projection ----------
            with (
                tc.tile_pool(name="osb", bufs=3) as p_o,
                tc.tile_pool(name="psout", bufs=4, space="PSUM") as ps_o,
            ):
                for tch in range(QN // 128):
                    po = ps_o.tile([128, F], f32, tag="po")
                    for fc in range(4):
                        nc.tensor.matmul(
                            po[:],
                            yT[fc][:, tch * 128:(tch + 1) * 128],
                            two[fc][:],
                            start=(fc == 0), stop=False)
                    nc.tensor.matmul(
                        po[:], ones1[:], tbo[:], start=False, stop=True)
                    osb = p_o.tile([128, F], f32, tag="osb")
                    nc.scalar.activation(osb[:], po[:], AF.Relu)
                    nc.sync.dma_start(
                        out_d[tch * 128:(tch + 1) * 128, :], osb[:])

    nc.compile()
    return nc


def _host_prep(inputs, S_=S, Sq_=None):
    """Build per-core input maps (host-side layout prep only)."""
    if Sq_ is None:
        Sq_ = S_ // 2
    q, k, v = inputs["q"], inputs["k"], inputs["v"]
    Wq, bq, Wk, bk = inputs["Wq"], inputs["bq"], inputs["Wk"], inputs["bk"]
    Wv, bv, Wo, bo = inputs["Wv"], inputs["bv"], inputs["Wo"], inputs["bo"]

    # rope de-interleave permutation per head: [evens, odds]
    j = np.arange(D)
    inner = np.where(j < D // 2, 2 * j, 2 * (j - D // 2) + 1)
    perm = (np.arange(H)[:, None] * D + inner[None, :]).reshape(-1)

    wqT = np.ascontiguousarray(Wq[perm, :].T)
    wkT = np.ascontiguousarray(Wk[perm, :].T)
    wvT = np.ascontiguousarray(Wv.T)
    woT = np.ascontiguousarray(Wo.T)
    bq_p = np.ascontiguousarray(bq[perm].reshape(F, 1))
    bk_p = np.ascontiguousarray(bk[perm].reshape(F, 1))
    bv_row = np.ascontiguousarray(bv.reshape(1, F))
    bo_row = np.ascontiguousarray(bo.reshape(1, F))

    # rope tables, (128, S): row p -> pair index p%32, sign -sin for
    # "real" 32-blocks, +sin for "imag" blocks
    i = np.arange(D // 2, dtype=np.float64)
    thetas = THETA ** (-2.0 * i / D)
    s_idx = np.arange(S_, dtype=np.float64)
    ang = s_idx[None, :] * thetas[:, None]          # (32, S)
    cos32, sin32 = np.cos(ang), np.sin(ang)
    p = np.arange(128)
    ropeC = cos32[p % 32, :].astype(np.float32)
    sgn = np.where((p % 64) < 32, -1.0, 1.0)[:, None]
    ropeS = (sgn * sin32[p % 32, :]).astype(np.float32)
    ropeC = np.ascontiguousarray(ropeC)
    ropeS = np.ascontiguousarray(ropeS)

    in_maps = []
    for c in range(NCORES):
        b, qh = c // 2, c % 2
        qs = slice(qh * Sq_, (qh + 1) * Sq_)
        m = {
            "xqT": np.ascontiguousarray(q[qs, b, :].T),
            "xkT": np.ascontiguousarray(k[:S_, b, :].T),
            "xvT": np.ascontiguousarray(v[:S_, b, :].T),
            "wqT": wqT, "wkT": wkT, "wvT": wvT, "woT": woT,
            "bq": bq_p, "bk": bk_p, "bv_row": bv_row, "bo_row": bo_row,
            "ropeC": ropeC, "ropeS": ropeS,
            "ropeCq": np.ascontiguousarray(ropeC[:, qs]),
            "ropeSq": np.ascontiguousarray(ropeS[:, qs]),
        }
        in_maps.append(m)
    return in_maps


def kernel(**inputs):
    from concourse import bass_utils

    key = "full"
    if key not in _CACHE:
        _CACHE[key] = _build()
    nc = _CACHE[key]

    in_maps = _host_prep(inputs)
    res = bass_utils.run_bass_kernel_spmd(
        nc, in_maps, core_ids=list(range(NCORES)))

    out = np.empty((S, B, F), dtype=np.float32)
    Sq_ = S // 2
    for c in range(NCORES):
        b, qh = c // 2, c % 2
        out[qh * Sq_:(qh + 1) * Sq_, b, :] = res.results[c]["out"]
    return out
